# revision 27
# baseline (speedup 1.0000x reference)
"""MultiHeadSelfAttention + residual + LayerNorm on 8 TRN2 NeuronCores.

Sharding: 2 cores per batch element (B=4), heads split 8/8 within the pair
(tensor parallel). Each core: QKV for its heads over the full sequence,
attention in 512-query chunks (packed head pairs, [V|ones] softmax-denominator
fold, f32r matmuls, one batched exp per key-tile covering both heads),
row-sharded output projection software-pipelined into the next chunk's
attention (the PE sits ~35% idle waiting on exp, so out-proj matmul groups
are emitted between kti iterations), fine-grained pairwise AllReduce per
256-row block and residual+LayerNorm right after each AR, all overlapped;
only the last 128-row block's AR+LN is exposed at the end.

LayerNorm's 1/std uses exp(-0.5*ln(var+eps)): ln and exp live in the same
ACT table set, so no ACT_TABLE_LOAD churn (Sqrt would force a reload around
every attention exp batch).

Softmax denominators are broadcast across partitions via a DRAM bounce
(SBUF-source partition-broadcast DMA is illegal, and gpsimd
partition_broadcast would serialize behind the blocking collectives on the
gpsimd queue). Reciprocals run on a [64, 8] scatter so all DVE lanes work.

Self-contained: shapes/sharding hardcoded; builds and caches the NEFF on
first call. Output y is the full batch row range; the host slices each
core's half.
"""
import numpy as np

import concourse.bass as bass
import concourse.tile as tile
from concourse import bacc, mybir
from concourse.bass_utils import run_bass_kernel_spmd
from concourse.masks import make_identity

F32 = mybir.dt.float32
F32R = mybir.dt.float32r
BF16 = mybir.dt.bfloat16

B, S, D, H, DEPTH = 4, 2048, 1024, 16, 64
HL = 8            # heads per core
EL = 512          # local e width (HL * DEPTH)
CT = 8            # c tiles (D / 128)
ST = 16           # s tiles (S / 128)
SC = 4            # s chunks (S / 512)
ET = 4            # local e tiles (EL / 128)
QC = 4            # query chunks in phase B (S / 512)
EPS = 1e-6
RG = [[0, 1], [2, 3], [4, 5], [6, 7]]

_CACHE = {}
_LAST_IN_MAPS = None


def _build():
    nc = bacc.Bacc("TRN2", target_bir_lowering=False, debug=False, num_devices=8)

    x_in = nc.dram_tensor("x", [S, D], F32R, kind="ExternalInput")
    wq_in = nc.dram_tensor("wq", [D, EL], F32R, kind="ExternalInput")
    wk_in = nc.dram_tensor("wk", [D, EL], F32R, kind="ExternalInput")
    wv_in = nc.dram_tensor("wv", [D, EL], F32R, kind="ExternalInput")
    wo_in = nc.dram_tensor("wo", [EL, D], F32R, kind="ExternalInput")
    bqk_in = nc.dram_tensor("bqk", [128, 2 * ET], F32, kind="ExternalInput")
    bv_in = nc.dram_tensor("bv", [1, EL], F32, kind="ExternalInput")
    bo_in = nc.dram_tensor("bo", [1, D], F32, kind="ExternalInput")
    gamma_in = nc.dram_tensor("gamma", [1, D], F32, kind="ExternalInput")
    beta_in = nc.dram_tensor("beta", [1, D], F32, kind="ExternalInput")
    y_out = nc.dram_tensor("y", [S, D], F32, kind="ExternalOutput")

    with tile.TileContext(nc) as tc:
        with tc.tile_pool(name="const", bufs=1) as const, \
             tc.tile_pool(name="dram", bufs=1, space="DRAM") as dram:

            ident_f = const.tile([128, 128], F32)
            make_identity(nc, ident_f[:])
            ident = const.tile([128, 128], F32R)
            nc.vector.tensor_copy(ident[:], ident_f[:])
            ones1 = const.tile([128, 1], F32)
            nc.gpsimd.memset(ones1[:], 1.0)
            eps_sb = const.tile([128, 1], F32)
            nc.gpsimd.memset(eps_sb[:], EPS)

            # AR blocks of 256 rows (one 512 block for the last chunk: a
            # single serial AR beats two at the cold-chip tail); bf16
            # payload halves collective bytes
            BLK_ROWS = [256] * 6 + [512]
            BLK_OFF = [sum(BLK_ROWS[:i]) for i in range(len(BLK_ROWS))]
            y_part = [dram.tile([r, D], BF16, name=f"y_part{i}")
                      for i, r in enumerate(BLK_ROWS)]
            # last block uses AllGather + local add (no CC-engine compute);
            # others AllReduce
            ar_out = [dram.tile([r, D], BF16, name=f"ar_out{i}")
                      for i, r in enumerate(BLK_ROWS[:-1])]
            ag_out = dram.tile([2, BLK_ROWS[-1], D], BF16, name="ag_out")
            den_d = dram.tile([QC * ET * 2, 1, 512], F32)
            rec_d = dram.tile([QC * ET * 2, 64, 8], F32)

            bqk_sb = const.tile([128, 2 * ET], F32)
            nc.sync.dma_start(bqk_sb[:], bqk_in.ap()[:])
            bv_bc = const.tile([128, EL], F32)
            nc.sync.dma_start(bv_bc[:], bv_in.ap().to_broadcast((128, EL)))

            with tc.tile_pool(name="qkv", bufs=1) as qkvp:
                kt = qkvp.tile([128, ET, S], F32R)                 # K^T [e, s]
                qt = qkvp.tile([128, ET, S], F32R)                 # Q^T [e, s]
                vt = qkvp.tile([128, ST, HL, DEPTH + 1], F32R)     # V natural + ones
                nc.vector.tensor_copy(vt[:, :, :, DEPTH:DEPTH + 1],
                                      ones1[:].to_broadcast((128, ST, HL, 1)))

                # ---- phase A: transpose X per chunk; project Q, K, V.
                # Transposes for chunk sc+1 are emitted BEFORE chunk sc's
                # matmuls (transpose-ahead) so the PE never waits on the
                # transpose->DVE-copy round trip at chunk boundaries; 4
                # transposes share one PSUM bank and drain in one strided
                # DVE copy. x-chunk DMAs precede the weight DMAs so the
                # first transposes start immediately.
                with tc.tile_pool(name="xnA", bufs=4) as xnA, \
                     tc.tile_pool(name="xtA", bufs=2) as xtA, \
                     tc.tile_pool(name="w3", bufs=1) as w3, \
                     tc.tile_pool(name="tpA", bufs=2, space="PSUM") as tpA, \
                     tc.tile_pool(name="psA", bufs=4, space="PSUM") as psA:

                    def emit_transposes(sc):
                        xt_c = xtA.tile([128, CT, 512], F32R, name="xt_c", tag="xt_c")
                        for sl in range(4):
                            si = 4 * sc + sl
                            xn = xnA.tile([128, D], F32R, name="xn", tag="xn")
                            nc.sync.dma_start(xn[:], x_in.ap()[128 * si:128 * (si + 1), :])
                            for g in range(2):
                                tp4 = tpA.tile([128, 512], F32R, name="tp4", tag="tp4")
                                for c4 in range(4):
                                    ci = 4 * g + c4
                                    nc.tensor.transpose(
                                        tp4[:, 128 * c4:128 * (c4 + 1)],
                                        xn[:, 128 * ci:128 * (ci + 1)], ident[:])
                                nc.vector.tensor_copy(
                                    xt_c[:, 4 * g:4 * (g + 1), 128 * sl:128 * (sl + 1)],
                                    tp4[:].rearrange("p (a b) -> p a b", a=4))
                        return xt_c

                    def emit_w_loads():
                        wsb = {}
                        for nm, wdram in (("q", wq_in), ("k", wk_in), ("v", wv_in)):
                            wsb[nm] = w3.tile([128, CT, EL], F32R, name=f"w{nm}")
                            for ci in range(CT):
                                nc.sync.dma_start(wsb[nm][:, ci, :],
                                                  wdram.ap()[128 * ci:128 * (ci + 1), :])
                        return wsb

                    xt_tiles = {0: emit_transposes(0)}
                    wsb = None
                    for sc in range(SC):
                        if sc + 1 < SC:
                            xt_tiles[sc + 1] = emit_transposes(sc + 1)
                        if wsb is None:
                            wsb = emit_w_loads()
                        cs = slice(512 * sc, 512 * (sc + 1))
                        xt_c = xt_tiles.pop(sc)
                        for dst, wname, bcol in ((qt, "q", 0), (kt, "k", ET)):
                            for j in range(ET):
                                ps = psA.tile([128, 512], F32, name="pqk", tag="pqk")
                                for ci in range(CT):
                                    nc.tensor.matmul(
                                        ps[:], wsb[wname][:, ci, 128 * j:128 * (j + 1)],
                                        xt_c[:, ci, :], start=(ci == 0), stop=(ci == CT - 1))
                                nc.vector.tensor_scalar_add(
                                    dst[:, j, cs], ps[:], bqk_sb[:, bcol + j:bcol + j + 1])
                        for sl in range(4):
                            si = 4 * sc + sl
                            ps = psA.tile([128, 512], F32, name="pv", tag="pqk")
                            for ci in range(CT):
                                nc.tensor.matmul(
                                    ps[:], xt_c[:, ci, 128 * sl:128 * (sl + 1)],
                                    wsb["v"][:, ci, :], start=(ci == 0), stop=(ci == CT - 1))
                            nc.vector.tensor_add(
                                vt[:, si, :, 0:DEPTH],
                                ps[:].rearrange("p (h e) -> p h e", h=HL),
                                bv_bc[:].rearrange("p (h e) -> p h e", h=HL))

                # ---- phase B: attention; out-proj/AR/LN pipelined into it ----
                with tc.tile_pool(name="wo", bufs=1) as wop, \
                     tc.tile_pool(name="lnc", bufs=1) as lnc, \
                     tc.tile_pool(name="atc", bufs=2) as atcp, \
                     tc.tile_pool(name="ep3", bufs=2) as ep3, \
                     tc.tile_pool(name="psb", bufs=3) as psb, \
                     tc.tile_pool(name="ysb", bufs=2) as ysb, \
                     tc.tile_pool(name="ln", bufs=2) as ln, \
                     tc.tile_pool(name="sps", bufs=2, space="PSUM") as sps, \
                     tc.tile_pool(name="aps", bufs=1, space="PSUM") as aps, \
                     tc.tile_pool(name="psO", bufs=1, space="PSUM") as psO:
                    wo_sb = wop.tile([128, ET, D], F32R)
                    for j in range(ET):
                        nc.sync.dma_start(wo_sb[:, j, :], wo_in.ap()[128 * j:128 * (j + 1), :])
                    bo_bc = lnc.tile([128, D], F32)
                    nc.sync.dma_start(bo_bc[:], bo_in.ap().to_broadcast((128, D)))
                    gam_bc = lnc.tile([128, D], F32)
                    nc.sync.dma_start(gam_bc[:], gamma_in.ap().to_broadcast((128, D)))
                    bet_bc = lnc.tile([128, D], F32)
                    nc.sync.dma_start(bet_bc[:], beta_in.ap().to_broadcast((128, D)))

                    def outproj_group(qc, stl, mh, a_prev):
                        """One [128 rows x 512 cols] out-proj partial for chunk qc."""
                        rloc = slice(128 * stl, 128 * (stl + 1))
                        ms = slice(512 * mh, 512 * (mh + 1))
                        ps = psO.tile([128, 512], F32, name="py",
                                      tag=f"po{(2 * stl + mh) % 2}")
                        for j in range(ET):
                            nc.tensor.matmul(ps[:], a_prev[:, j, rloc],
                                             wo_sb[:, j, ms],
                                             start=(j == 0), stop=(j == ET - 1))
                        y_sb = ysb.tile([128, 512], BF16, name="y_sb", tag="y_sb")
                        # bo/2 folded here (host halves bo) so LN skips its add
                        nc.vector.tensor_add(y_sb[:], ps[:], bo_bc[:, ms])
                        # global row = 512*qc + 128*stl; find block + offset
                        grow = 512 * qc + 128 * stl
                        bi = max(i for i in range(len(BLK_OFF)) if BLK_OFF[i] <= grow)
                        nc.sync.dma_start(
                            y_part[bi][grow - BLK_OFF[bi]:grow - BLK_OFF[bi] + 128, ms],
                            y_sb[:])

                    def ar_block(bi):
                        if bi == len(BLK_ROWS) - 1:
                            nc.gpsimd.collective_compute(
                                "AllGather", mybir.AluOpType.bypass,
                                replica_groups=RG,
                                ins=[y_part[bi].opt()], outs=[ag_out.opt()])
                        else:
                            nc.gpsimd.collective_compute(
                                "AllReduce", mybir.AluOpType.add,
                                replica_groups=RG,
                                ins=[y_part[bi].opt()], outs=[ar_out[bi].opt()])

                    def ln_block(bi):
                        """Residual + LayerNorm for all 128-row tiles of block bi.

                        bo/2 is pre-folded into each core's out-proj partial on
                        the host, so no separate bias add is needed here.
                        """
                        last = bi == len(BLK_ROWS) - 1
                        for rt in range(BLK_ROWS[bi] // 128):
                            grow = slice(BLK_OFF[bi] + 128 * rt,
                                         BLK_OFF[bi] + 128 * (rt + 1))
                            rs = slice(128 * rt, 128 * (rt + 1))
                            tb = ln.tile([128, D], BF16, name="tb", tag="tb")
                            nc.sync.dma_start(
                                tb[:], ag_out[0, rs, :] if last else ar_out[bi][rs, :])
                            r = ln.tile([128, D], F32, name="r", tag="r")
                            nc.sync.dma_start(r[:], x_in.ap()[grow, :].bitcast(F32))
                            t = ln.tile([128, D], F32, name="t", tag="t")
                            nc.vector.tensor_add(t[:], r[:], tb[:])
                            if last:
                                tb2 = ln.tile([128, D], BF16, name="tb2", tag="tb2")
                                nc.sync.dma_start(tb2[:], ag_out[1, rs, :])
                                nc.vector.tensor_add(t[:], t[:], tb2[:])
                            stats = ln.tile([128, 2, 6], F32, name="stats", tag="stats")
                            tv = t[:].rearrange("p (a b) -> p a b", a=2)
                            for sub in range(2):
                                nc.vector.bn_stats(stats[:, sub, :], tv[:, sub, :])
                            mv = ln.tile([128, 2], F32, name="mv", tag="mv")
                            nc.vector.bn_aggr(mv[:], stats[:])
                            # rstd = 1/sqrt(var+eps) on DVE (Newton from 1/v
                            # seed) so ACT runs exp only -> one table load ever
                            vv = ln.tile([128, 1], F32, name="vv", tag="vv")
                            nc.vector.tensor_scalar_add(vv[:], mv[:, 1:2], EPS)
                            rstd = ln.tile([128, 1], F32, name="rstd", tag="rstd")
                            nc.vector.reciprocal(rstd[:], vv[:])
                            yt = ln.tile([128, 1], F32, name="yt", tag="yt")
                            for _ in range(2):
                                nc.vector.tensor_mul(yt[:], rstd[:], rstd[:])
                                nc.vector.tensor_mul(yt[:], yt[:], vv[:])
                                nc.vector.tensor_scalar(
                                    yt[:], yt[:], -0.5, 1.5,
                                    mybir.AluOpType.mult, mybir.AluOpType.add)
                                nc.vector.tensor_mul(rstd[:], rstd[:], yt[:])
                            o = ln.tile([128, D], F32, name="o", tag="o")
                            nc.vector.tensor_scalar(
                                o[:], t[:], mv[:, 0:1], rstd[:],
                                mybir.AluOpType.subtract, mybir.AluOpType.mult)
                            nc.vector.tensor_mul(o[:], o[:], gam_bc[:])
                            nc.vector.tensor_add(o[:], o[:], bet_bc[:])
                            nc.sync.dma_start(y_out.ap()[grow, :], o[:])

                    # blocks whose row-range ends exactly at the given chunk
                    # half boundary (the 512 tail block triggers at half=1)
                    def blocks_of(pq, half):
                        end = 512 * pq + 256 * (half + 1)
                        return [bi for bi in range(len(BLK_ROWS))
                                if BLK_OFF[bi] + BLK_ROWS[bi] == end]

                    # per (qc, j): out-proj groups of chunk qc-1 fill PE idle
                    # slots; ARs trigger as soon as a block's rows are written;
                    # each block's LN lags its AR by two j-iterations (~45us,
                    # comfortably past AR completion) so LN's waits never
                    # stall the in-order queues yet little LN piles up at the
                    # end
                    def pending_pieces(qc, j):
                        ops = []
                        if qc >= 1:
                            ops += [("op", qc - 1, j, 0), ("op", qc - 1, j, 1)]
                            if j in (1, 3):
                                ops += [("ar", b) for b in blocks_of(qc - 1, j // 2)]
                        if j == 3 and qc >= 1:
                            ops += [("ln", b) for b in blocks_of(qc - 1, 0)]
                        if j == 1 and qc >= 2:
                            ops += [("ln", b) for b in blocks_of(qc - 2, 1)]
                        return ops

                    def run_piece(p):
                        if p[0] == "op":
                            _, pq, stl, mh = p
                            outproj_group(pq, stl, mh, a_prev)
                        elif p[0] == "ar":
                            ar_block(p[1])
                        else:
                            ln_block(p[1])

                    a_prev = None
                    for qc in range(QC):
                        qs = slice(512 * qc, 512 * (qc + 1))
                        a_t = atcp.tile([128, ET, 512], F32R, name="a_t", tag="a_t")
                        for j in range(ET):
                            pieces = pending_pieces(qc, j)
                            accs = [aps.tile([DEPTH + 1, 512], F32, name=f"acc{h}",
                                             tag=f"acc{h}")
                                    for h in range(2)]

                            def emit_pv(kp, pp):
                                for h01 in range(2):
                                    nc.tensor.matmul(
                                        accs[h01][:],
                                        vt[:, kp, 2 * j + h01, :],
                                        pp[:, 512 * h01:512 * (h01 + 1)],
                                        start=(kp == 0), stop=(kp == ST - 1))

                            p_prev = None
                            for kti in range(ST):
                                ks = slice(128 * kti, 128 * (kti + 1))
                                # scores first: they only depend on exp(kti-2)
                                # via sp buffer reuse, so exp(kti) can chase
                                # exp(kti-1) with no PE round-trip in between;
                                # pv(kti-1) runs in exp(kti)'s shadow.
                                # both heads' scores into one 2-bank PSUM tile
                                # so a single batched exp covers them
                                sp = sps.tile([128, 1024], F32, name="sp", tag="sp")
                                for h01 in range(2):
                                    rows = slice(64 * h01, 64 * (h01 + 1))
                                    nc.tensor.matmul(sp[:, 512 * h01:512 * (h01 + 1)],
                                                     kt[rows, j, ks], qt[rows, j, qs],
                                                     start=True, stop=True)
                                pp = psb.tile([128, 1024], F32R, name="pp", tag="pp")
                                nc.scalar.activation(pp[:], sp[:],
                                                     mybir.ActivationFunctionType.Exp,
                                                     scale=0.125)
                                if p_prev is not None:
                                    emit_pv(kti - 1, p_prev)
                                p_prev = pp
                                # pipelined out-proj/AR/LN pieces fill PE idle
                                if kti == 5 and pieces:
                                    run_piece(pieces[0])
                                if kti == 11 and len(pieces) > 1:
                                    run_piece(pieces[1])
                            emit_pv(ST - 1, p_prev)
                            for h01 in range(2):
                                idx = (qc * ET + j) * 2 + h01
                                acc_sb = ep3.tile([DEPTH + 1, 512], F32, name="acc_sb",
                                                  tag="acc_sb")
                                nc.vector.tensor_copy(acc_sb[:], accs[h01][:])
                                nc.sync.dma_start(den_d[idx],
                                                  acc_sb[DEPTH:DEPTH + 1, :])
                                rin = ep3.tile([64, 8], F32, name="rin", tag="rin")
                                nc.sync.dma_start(rin[:], den_d[idx].rearrange(
                                    "a (p f) -> (a p) f", p=64))
                                nc.vector.reciprocal(rin[:], rin[:])
                                nc.sync.dma_start(rec_d[idx], rin[:])
                                rbc = ep3.tile([64, 512], F32, name="rbc", tag="rbc")
                                rsrc = rec_d[idx]
                                nc.sync.dma_start(
                                    rbc[:],
                                    bass.AP(tensor=rsrc.tensor, offset=rsrc.offset,
                                            ap=[[0, 64], [1, 512]]))
                                if h01 == 0:
                                    nc.vector.tensor_mul(a_t[0:64, j, :],
                                                         acc_sb[0:DEPTH, :], rbc[:])
                                else:
                                    nrm = ep3.tile([64, 512], F32R, name="nrm", tag="nrm")
                                    nc.vector.tensor_mul(nrm[:], acc_sb[0:DEPTH, :], rbc[:])
                                    nc.sync.dma_start(a_t[64:128, j, :], nrm[:])
                            # AR/LN pieces after the epilogue so the acc-release
                            # copy isn't queued behind LN's DVE work
                            for p in pieces[2:]:
                                run_piece(p)
                        a_prev = a_t
                    # drain: out-proj + AllGather for the last chunk; chunk
                    # 2's second-half LN runs while they proceed, then the
                    # last block's LN closes after the gather
                    for j in range(ET):
                        outproj_group(QC - 1, j, 0, a_prev)
                        outproj_group(QC - 1, j, 1, a_prev)
                        if j in (1, 3):
                            for b in blocks_of(QC - 1, j // 2):
                                ar_block(b)
                    for b in blocks_of(QC - 2, 1):
                        ln_block(b)
                    for half in range(2):
                        for b in blocks_of(QC - 1, half):
                            ln_block(b)

    nc.compile()
    return nc


def kernel(inputs, Wq, bq, Wk, bk, Wv, bv, Wo, bo, gamma, beta):
    if "nc" not in _CACHE:
        _CACHE["nc"] = _build()
    nc = _CACHE["nc"]

    inputs = np.ascontiguousarray(np.asarray(inputs, dtype=np.float32))
    Wq = np.asarray(Wq, np.float32); Wk = np.asarray(Wk, np.float32)
    Wv = np.asarray(Wv, np.float32); Wo = np.asarray(Wo, np.float32)
    bq = np.asarray(bq, np.float32); bk = np.asarray(bk, np.float32)
    bv = np.asarray(bv, np.float32); bo = np.asarray(bo, np.float32)
    gamma = np.asarray(gamma, np.float32); beta = np.asarray(beta, np.float32)

    in_maps = []
    for c in range(8):
        b, hf = c // 2, c % 2
        es = slice(EL * hf, EL * (hf + 1))
        bqk = np.concatenate([bq[es].reshape(ET, 128).T, bk[es].reshape(ET, 128).T],
                             axis=1)
        in_maps.append({
            "x": inputs[b],
            "wq": np.ascontiguousarray(Wq[:, es]),
            "wk": np.ascontiguousarray(Wk[:, es]),
            "wv": np.ascontiguousarray(Wv[:, es]),
            "wo": np.ascontiguousarray(Wo[es, :]),
            "bqk": np.ascontiguousarray(bqk),
            "bv": bv[es].reshape(1, EL).copy(),
            "bo": (bo / 2.0).reshape(1, D).copy(),
            "gamma": gamma.reshape(1, D).copy(),
            "beta": beta.reshape(1, D).copy(),
        })

    global _LAST_IN_MAPS
    _LAST_IN_MAPS = in_maps
    res = run_bass_kernel_spmd(nc, in_maps, core_ids=list(range(8)))

    out = np.empty((B, S, D), dtype=np.float32)
    for c in range(8):
        b, hf = c // 2, c % 2
        out[b, 1024 * hf:1024 * (hf + 1)] = res.results[c]["y"][1024 * hf:1024 * (hf + 1)]
    return out
